# revision 3
# baseline (speedup 1.0000x reference)
"""Trainium2 Bass kernel for nn_ExpandEvecs.

Reference computation (fp32):
    evecs [B=4, C=1, N=1024, K=16]
    outers[b,k,c,n,m] = evecs[b,c,n,k] * evecs[b,c,m,k]
    cube = cumsum(outers, axis=k)          -> [B, K, C, N, N]
    out  = cube.reshape(B, K*C, N, N)      -> [4, 16, 1024, 1024]

i.e. out[b, k] = X[:, :k+1] @ X[:, :k+1]^T with X = evecs[b, 0]  [N, K].

Key optimizations vs a full fp32 writeout (tolerance is rel_err < 2e-2,
we land ~3e-4):
  1. fp16 output: halves HBM write traffic. Host upcasts to fp32.
  2. Symmetry: out[b,k] is symmetric, so only the upper block-triangle
     (36 of 64 [128,128] blocks per slab, 56.25%) is computed + written.
     Host mirrors the strictly-lower blocks via transpose.
  3. Single-pass fp16 matmul (no hi/lo split): X_h (x) X_h in fp32 PSUM
     gives ~1e-3 elementwise error, far inside the gate.

Sharding: 8 cores, core c -> (b = c//2, level-half = c%2); 8 levels per
core. The level subset is encoded in the DATA (per-level masked fp16
moving operand xm), so the SPMD program is identical on all cores.

Structure ("chunk" variant): row-chunk-major so all matmuls of a chunk
share one stationary operand (amortizing LDWEIGHTS, which cost 146 ns
each = 14 us for 96 loads in the level-major variant). Per chunk i
(trapezoid width w_i = 1024-128*i), the 8 levels' [128, w_i] pieces are
computed into a level-concatenated stage tile [128, 8*w_i] fp16 and
drained by ONE DMA. Chunks run smallest-first so the first DMA fires
within ~1 us; copies for a chunk all go to one engine (alternating
DVE/ACT across chunks) so each DMA waits on a single engine semaphore.
Per-core writes: 9 MiB -> ~26 us at the ~360 GB/s DMA roofline.
"""

import sys

if "/opt/trn_rl_repo" not in sys.path:
    sys.path.insert(0, "/opt/trn_rl_repo")

import numpy as np

B = 4          # batch
NLEV = 16      # total levels (K)
N = 1024       # vector length
KC = 16        # contract dim (= K)
NCORES = 8
LEV = 8        # levels per core
P = 128        # partition tile (row chunk)
RC = N // P    # 8 row chunks
FH = 512       # max matmul moving free dim / psum bank cols (fp32)

CHUNKW = [N - P * i for i in range(RC)]          # trapezoid widths
CHUNKOFF = [LEV * sum(CHUNKW[:i]) for i in range(RC)]  # col offset in flat out
TOTW = LEV * sum(CHUNKW)                          # 36864
# smallest chunk first: earliest DMA + monotonically increasing unit size
CHUNK_ORDER = list(range(RC - 1, -1, -1))
# chunk -> copy engine ("v"=DVE, "a"=ACT), balancing total cols
CHUNK_ENG = {0: "a", 1: "v", 2: "a", 3: "v", 4: "a", 5: "v", 6: "a", 7: "v"}

_nc_cache = {}


def build_bass(variant="chunk"):
    import concourse.mybir as mybir
    import concourse.tile as tile
    from concourse import bacc

    dt = mybir.dt
    nc = bacc.Bacc(None, target_bir_lowering=False)
    xr = nc.dram_tensor("xr", [KC, N], dt.float16, kind="ExternalInput")
    xm = nc.dram_tensor("xm", [KC, LEV * N], dt.float16, kind="ExternalInput")
    out = nc.dram_tensor("out", [P, TOTW], dt.float16, kind="ExternalOutput")

    with tile.TileContext(nc) as tc:
        with (
            tc.tile_pool(name="xin", bufs=1) as xin,
            tc.tile_pool(name="stage", bufs=1) as stg,
            tc.tile_pool(name="ps", bufs=8, space="PSUM") as psp,
        ):
            xr_t = xin.tile([KC, N], dt.float16, tag="xr")
            nc.sync.dma_start(xr_t[:], xr[:])
            xm_t = xin.tile([KC, LEV * N], dt.float16, tag="xm")
            nc.sync.dma_start(xm_t[:], xm[:])

            for i in CHUNK_ORDER:
                w = CHUNKW[i]
                eng = CHUNK_ENG[i]
                sc = stg.tile([P, LEV * w], dt.float16, tag=f"sc{i}")
                lhsT = xr_t[:, i * P:(i + 1) * P]
                for j in range(LEV):
                    cs = i * P
                    # pieces of <=512 cols within this level's trapezoid
                    o = 0
                    while o < w:
                        pw = min(FH, w - o)
                        ps = psp.tile([P, FH], dt.float32, tag="ps")
                        nc.tensor.matmul(
                            ps[:, :pw],
                            lhsT,
                            xm_t[:, j * N + cs + o:j * N + cs + o + pw],
                            start=True,
                            stop=True,
                        )
                        dst = sc[:, j * w + o:j * w + o + pw]
                        if eng == "v":
                            nc.vector.tensor_copy(dst, ps[:, :pw])
                        else:
                            nc.scalar.copy(dst, ps[:, :pw])
                        o += pw
                nc.sync.dma_start(
                    out[:, CHUNKOFF[i]:CHUNKOFF[i] + LEV * w], sc[:])
    nc.compile()
    return nc


def _get_nc(variant):
    if variant not in _nc_cache:
        _nc_cache[variant] = build_bass(variant)
    return _nc_cache[variant]


def host_inputs(evecs, variant="chunk"):
    """Per-core input maps. Core c -> (b=c//2, half=c%2)."""
    in_maps = []
    for c in range(NCORES):
        b, half = divmod(c, 2)
        X = np.asarray(evecs[b, 0], dtype=np.float32)      # [1024, 16]
        xr16 = np.ascontiguousarray(X.T).astype(np.float16)  # [16, 1024]
        xm16 = np.zeros((KC, LEV, N), np.float16)
        for j in range(LEV):
            kmax = half * LEV + j + 1   # number of live eigvecs at level
            xm16[:kmax, j, :] = xr16[:kmax]
        in_maps.append({
            "xr": xr16,
            "xm": np.ascontiguousarray(xm16.reshape(KC, LEV * N)),
        })
    return in_maps


def unpack(results):
    """Assemble the full fp32 output from per-core packed fp16 buffers."""
    full = np.empty((B, NLEV, N, N), np.float32)
    for c in range(NCORES):
        b, half = divmod(c, 2)
        flat = results[c]["out"]   # [128, TOTW] fp16
        for i in range(RC):
            w = CHUNKW[i]
            o = CHUNKOFF[i]
            # [128, LEV, w] view of chunk i, level-concatenated
            blk = flat[:, o:o + LEV * w].reshape(P, LEV, w)
            for j in range(LEV):
                slab = full[b, half * LEV + j]
                slab[i * P:(i + 1) * P, i * P:] = blk[:, j, :]
    # mirror the strictly-lower blocks from the upper triangle
    V = full.reshape(B, NLEV, RC, P, RC, P)
    for i2 in range(RC):
        for j2 in range(i2):
            V[:, :, i2, :, j2, :] = V[:, :, j2, :, i2, :].swapaxes(-2, -1)
    return full


def run(evecs, trace=False, mm_dtype="chunk", **spmd_kwargs):
    from concourse.bass_utils import run_bass_kernel_spmd

    variant = "chunk"
    nc = _get_nc(variant)
    in_maps = host_inputs(evecs, variant)
    r = run_bass_kernel_spmd(
        nc, in_maps, core_ids=list(range(NCORES)), trace=trace, **spmd_kwargs
    )
    return unpack(r.results), r


def kernel(**inputs):
    evecs = np.asarray(inputs["evecs"])
    full, _ = run(evecs)
    return full


# revision 5
# speedup vs baseline: 1.0141x; 1.0141x over previous
"""Trainium2 Bass kernel for nn_ExpandEvecs.

Reference computation (fp32):
    evecs [B=4, C=1, N=1024, K=16]
    outers[b,k,c,n,m] = evecs[b,c,n,k] * evecs[b,c,m,k]
    cube = cumsum(outers, axis=k)          -> [B, K, C, N, N]
    out  = cube.reshape(B, K*C, N, N)      -> [4, 16, 1024, 1024]

i.e. out[b, k] = X[:, :k+1] @ X[:, :k+1]^T with X = evecs[b, 0]  [N, K].

Key optimizations vs a full fp32 writeout (tolerance is rel_err < 2e-2,
we land ~3e-4):
  1. fp16 output: halves HBM write traffic. Host upcasts to fp32.
  2. Symmetry: out[b,k] is symmetric, so only the upper block-triangle
     (36 of 64 [128,128] blocks per slab, 56.25%) is computed + written.
     Host mirrors the strictly-lower blocks via transpose.
  3. Single-pass fp16 matmul (no hi/lo split): X_h (x) X_h in fp32 PSUM
     gives ~1e-3 elementwise error, far inside the gate.

Sharding: 8 cores, core c -> (b = c//2, level-half = c%2); 8 levels per
core. The level subset is encoded in the DATA (per-level masked fp16
moving operand xm), so the SPMD program is identical on all cores.

Structure: row-chunk-major (all matmuls of a chunk share one stationary
operand). Chunk i covers trapezoid cols [128*i, 1024) (width w_i) for
its 8 levels, staged in a [128, 8, w_i] fp16 tile. Each level's piece
is split at col 512: the 512-wide part is copied PSUM->SBUF by the
Vector engine, the remainder by the Scalar engine (chunk 3 flipped so
both engines get exactly 18432 cols); narrow chunks (w<=512) alternate
whole levels between the engines. Each chunk drains with TWO DMAs (one
per engine's region) so every DMA waits on a single engine semaphore.
Chunk order [5,3,1,0,2,4,6,7]: small first (early DMA start), big in
the middle, smallest last (short tail). Input xm is split across the
sync + gpsimd DGE queues to hide DMA latency at kernel start.
Per-core writes: 9 MiB -> ~26 us at the ~360 GB/s DMA roofline.
"""

import sys

if "/opt/trn_rl_repo" not in sys.path:
    sys.path.insert(0, "/opt/trn_rl_repo")

import numpy as np

B = 4          # batch
NLEV = 16      # total levels (K)
N = 1024       # vector length
KC = 16        # contract dim (= K)
NCORES = 8
LEV = 8        # levels per core
P = 128        # partition tile (row chunk)
RC = N // P    # 8 row chunks
FH = 512       # max matmul moving free dim / psum bank cols (fp32)

CHUNKW = [N - P * i for i in range(RC)]   # trapezoid widths 1024..128
# small first (early DMA), big middle, smallest last (short tail)
CHUNK_ORDER = [5, 3, 1, 0, 2, 4, 6, 7]
# wide chunks whose <=512 remainder goes to DVE instead of ACT (balance)
FLIPPED = {3}

_nc_cache = {}


def build_bass(variant="chunk2"):
    import concourse.mybir as mybir
    import concourse.tile as tile
    from concourse import bacc

    dt = mybir.dt
    nc = bacc.Bacc(None, target_bir_lowering=False)
    xr = nc.dram_tensor("xr", [KC, N], dt.float16, kind="ExternalInput")
    xm = nc.dram_tensor("xm", [KC, LEV * N], dt.float16, kind="ExternalInput")
    outs = []
    for i in range(RC):
        w = CHUNKW[i]
        if w > FH:
            shape = [P, LEV, w]
        else:
            shape = [P, 2, LEV // 2, w]   # [parity, level-within-parity]
        outs.append(nc.dram_tensor(f"out{i}", shape, dt.float16,
                                   kind="ExternalOutput"))

    with tile.TileContext(nc) as tc:
        with (
            tc.tile_pool(name="xin", bufs=1) as xin,
            tc.tile_pool(name="stage", bufs=1) as stg,
            tc.tile_pool(name="ps", bufs=8, space="PSUM") as psp,
        ):
            xm_t = xin.tile([KC, LEV * N], dt.float16, tag="xm")
            # level-0 slice on the sync queue (needed first), the rest in
            # parallel on the gpsimd DGE queue to hide DMA latency
            nc.sync.dma_start(xm_t[:, 0:N], xm[:, 0:N])
            nc.gpsimd.dma_start(xm_t[:, N:], xm[:, N:])
            xr_t = xin.tile([KC, N], dt.float16, tag="xr")
            nc.sync.dma_start(xr_t[:], xr[:])

            def vcopy(dst, src):
                nc.vector.tensor_copy(dst, src)

            def acopy(dst, src):
                nc.scalar.copy(dst, src)

            for i in CHUNK_ORDER:
                w = CHUNKW[i]
                sc = stg.tile(
                    [P, LEV, w] if w > FH else [P, 2, LEV // 2, w],
                    dt.float16, tag=f"sc{i}")
                lhsT = xr_t[:, i * P:(i + 1) * P]
                cs = i * P
                for j in range(LEV):
                    if w > FH:
                        pieces = [(0, FH, "a" if i in FLIPPED else "v"),
                                  (FH, w - FH, "v" if i in FLIPPED else "a")]
                    else:
                        pieces = [(0, w, "v" if j % 2 == 0 else "a")]
                    for o, pw, eng in pieces:
                        ps = psp.tile([P, FH], dt.float32, tag="ps")
                        nc.tensor.matmul(
                            ps[:, :pw],
                            lhsT,
                            xm_t[:, j * N + cs + o:j * N + cs + o + pw],
                            start=True,
                            stop=True,
                        )
                        if w > FH:
                            dst = sc[:, j, o:o + pw]
                        else:
                            dst = sc[:, j % 2, j // 2, :]
                        (vcopy if eng == "v" else acopy)(dst, ps[:, :pw])
                # two drains, each gated on a single engine's copies
                if w > FH:
                    lo, hi = sc[:, :, 0:FH], sc[:, :, FH:w]
                    dlo, dhi = outs[i][:, :, 0:FH], outs[i][:, :, FH:w]
                    nc.sync.dma_start(dlo, lo)
                    nc.sync.dma_start(dhi, hi)
                else:
                    nc.sync.dma_start(outs[i][:, 0], sc[:, 0])
                    nc.sync.dma_start(outs[i][:, 1], sc[:, 1])
    nc.compile()
    return nc


def _get_nc(variant):
    if variant not in _nc_cache:
        _nc_cache[variant] = build_bass(variant)
    return _nc_cache[variant]


def host_inputs(evecs, variant="chunk2"):
    """Per-core input maps. Core c -> (b=c//2, half=c%2)."""
    in_maps = []
    for c in range(NCORES):
        b, half = divmod(c, 2)
        X = np.asarray(evecs[b, 0], dtype=np.float32)      # [1024, 16]
        xr16 = np.ascontiguousarray(X.T).astype(np.float16)  # [16, 1024]
        xm16 = np.zeros((KC, LEV, N), np.float16)
        for j in range(LEV):
            kmax = half * LEV + j + 1   # number of live eigvecs at level
            xm16[:kmax, j, :] = xr16[:kmax]
        in_maps.append({
            "xr": xr16,
            "xm": np.ascontiguousarray(xm16.reshape(KC, LEV * N)),
        })
    return in_maps


def unpack(results):
    """Assemble the full fp32 output from per-core packed fp16 buffers."""
    full = np.empty((B, NLEV, N, N), np.float32)
    for c in range(NCORES):
        b, half = divmod(c, 2)
        for i in range(RC):
            w = CHUNKW[i]
            blk = results[c][f"out{i}"]
            if w <= FH:
                # [P, 2, LEV//2, w]: level j stored at [:, j%2, j//2];
                # transpose to [P, j//2, j%2, w] so the flat index
                # 2*(j//2) + (j%2) == j
                blk = blk.transpose(0, 2, 1, 3).reshape(P, LEV, w)
            for j in range(LEV):
                slab = full[b, half * LEV + j]
                slab[i * P:(i + 1) * P, i * P:] = blk[:, j, :]
    # mirror the strictly-lower blocks from the upper triangle
    V = full.reshape(B, NLEV, RC, P, RC, P)
    for i2 in range(RC):
        for j2 in range(i2):
            V[:, :, i2, :, j2, :] = V[:, :, j2, :, i2, :].swapaxes(-2, -1)
    return full


def run(evecs, trace=False, mm_dtype="chunk2", **spmd_kwargs):
    from concourse.bass_utils import run_bass_kernel_spmd

    variant = "chunk2"
    nc = _get_nc(variant)
    in_maps = host_inputs(evecs, variant)
    r = run_bass_kernel_spmd(
        nc, in_maps, core_ids=list(range(NCORES)), trace=trace, **spmd_kwargs
    )
    return unpack(r.results), r


def kernel(**inputs):
    evecs = np.asarray(inputs["evecs"])
    full, _ = run(evecs)
    return full


# revision 8
# speedup vs baseline: 1.0634x; 1.0486x over previous
"""Trainium2 Bass kernel for nn_ExpandEvecs.

Reference computation (fp32):
    evecs [B=4, C=1, N=1024, K=16]
    out[b, k] = X[:, :k+1] @ X[:, :k+1]^T, X = evecs[b, 0]  [N, K]
    -> [4, 16, 1024, 1024] fp32.

Optimizations (correctness gate is rel_err < 2e-2; we land ~5e-4):
  1. fp16 output (host upcasts) — halves the HBM write traffic.
  2. Symmetry — only the upper block-triangle (56.25%) is computed and
     written; the host mirrors the rest. Per-core writes 9 MiB -> ~26 us
     at the ~360 GB/s DMA roofline.
  3. Single-pass fp16 matmuls (no hi/lo split).
  4. The PE streams 0.83 ns/col (1.2 GHz); with 36864 cols/core it would
     be the bottleneck, so the narrow chunks 4-7 are computed as rank-1
     cumsum chains on the Vector engine instead:
         sc[j] = sc[j-1] + y_j (x) x_j
     via tensor_scalar_mul (4x fp16 mode) + tensor_add (2x fp16 mode),
     using an on-chip y broadcast (4 partition seeds + stream_shuffle).

Sharding: 8 cores, core c -> (b = c//2, level-half = c%2); the level
subset is encoded in the data (masked xm, yb, xc), so the SPMD program
is identical on all cores.

Schedule: chains (chunks 7,6,5,4) run first on DVE — small DMAs start
early; matmul chunks in order 2,0,1,3 with PSUM [128,1024] tiles, one
copy per chunk-level (ACT for c0/c2, DVE for c1/c3 after the chains),
one DMA per chunk gated on a single engine semaphore.
"""

import sys

if "/opt/trn_rl_repo" not in sys.path:
    sys.path.insert(0, "/opt/trn_rl_repo")

import numpy as np

B = 4          # batch
NLEV = 16      # total levels (K)
N = 1024       # vector length
KC = 16        # contract dim (= K)
NCORES = 8
LEV = 8        # levels per core
P = 128        # partition tile (row chunk)
RC = N // P    # 8 row chunks
FH = 512       # psum bank cols fp32 / max matmul moving free dim

CHUNKW = [N - P * i for i in range(RC)]   # trapezoid widths 1024..128
CHAIN_CHUNKS = [7, 6, 5, 4]               # rank-1 chains on DVE, in order
MM_CHUNKS = [2, 0, 1, 3]                  # matmul chunks, in order
MM_ENG = {0: "a", 2: "a", 1: "v", 3: "v"}

_nc_cache = {}


def build_bass(variant="chain4"):
    import concourse.mybir as mybir
    import concourse.tile as tile
    from concourse import bacc

    dt = mybir.dt
    nc = bacc.Bacc(None, target_bir_lowering=False)
    xr = nc.dram_tensor("xr", [KC, N], dt.float16, kind="ExternalInput")
    xm = nc.dram_tensor("xm", [KC, LEV * N], dt.float16, kind="ExternalInput")
    # y rows for slab cols [512:1024), level-major: yb[0, j*FH + (c-512)]
    yb = nc.dram_tensor("yb", [1, LEV * FH], dt.float16, kind="ExternalInput")
    # per-partition chain scalars: xc[p, (i-4)*LEV + j] = X[i*128+p, g_j]
    xc = nc.dram_tensor("xc", [P, 4 * LEV], dt.float32, kind="ExternalInput")
    outs = {}
    for i in range(RC):
        outs[i] = nc.dram_tensor(f"out{i}", [P, LEV, CHUNKW[i]], dt.float16,
                                 kind="ExternalOutput")

    with tile.TileContext(nc) as tc:
        with (
            tc.tile_pool(name="xin", bufs=1) as xin,
            tc.tile_pool(name="stage", bufs=1) as stg,
            tc.tile_pool(name="tmp", bufs=3) as tmpp,
            tc.tile_pool(name="ps", bufs=3, space="PSUM") as psp,
        ):
            xr_t = xin.tile([KC, N], dt.float16, tag="xr")
            nc.sync.dma_start(xr_t[:], xr[:])
            xm_t = xin.tile([KC, LEV * N], dt.float16, tag="xm")
            nc.sync.dma_start(xm_t[:], xm[:])
            # chain inputs ride the gpsimd DGE queue (parallel latency)
            ybq = xin.tile([P, LEV, FH], dt.float16, tag="ybq")
            for q in range(4):
                nc.gpsimd.dma_start(ybq[q * 32:q * 32 + 1, :, :], yb[:])
            xc_t = xin.tile([P, 4 * LEV], dt.float32, tag="xc")
            nc.gpsimd.dma_start(xc_t[:], xc[:])

            # y broadcast [128, LEV, 512] built incrementally (new col
            # span per chain chunk), replicated from partitions 0/32/64/96
            ybb = xin.tile([P, LEV, FH], dt.float16, tag="ybb")

            stages = {}
            for i in range(RC):
                stages[i] = stg.tile([P, LEV, CHUNKW[i]], dt.float16,
                                     tag=f"sc{i}", name=f"sc{i}")

            # ---- chain chunks (DVE) ----
            done_lo = FH   # cols [done_lo, 512) of ybb already built
            for i in CHAIN_CHUNKS:
                w = CHUNKW[i]
                lo = (i * P) - FH          # chunk cols in ybb coords
                if lo < done_lo:
                    nc.vector.stream_shuffle(
                        ybb[:, :, lo:done_lo], ybq[:, :, lo:done_lo],
                        [0] * 32)
                    done_lo = lo
                sc = stages[i]
                ci = (i - 4) * LEV
                for j in range(LEV):
                    scl = xc_t[:, ci + j:ci + j + 1]
                    if j == 0:
                        # masked matmul seed: level g_0 is cumulative over
                        # all eigvecs below this core's range, not rank-1
                        ps = psp.tile([P, 2 * FH], dt.float32, tag="ps")
                        nc.tensor.matmul(
                            ps[:, :w],
                            xr_t[:, i * P:(i + 1) * P],
                            xm_t[:, i * P:i * P + w],
                            start=True,
                            stop=True,
                        )
                        nc.vector.tensor_copy(sc[:, 0, :], ps[:, :w])
                    else:
                        tmp = tmpp.tile([P, FH], dt.float16, tag="tmp")
                        nc.vector.tensor_scalar_mul(
                            tmp[:, :w], ybb[:, j, lo:lo + w], scl)
                        nc.vector.tensor_add(
                            sc[:, j, :], sc[:, j - 1, :], tmp[:, :w])
                nc.sync.dma_start(outs[i][:], sc[:])

            # ---- matmul chunks ----
            for i in MM_CHUNKS:
                w = CHUNKW[i]
                sc = stages[i]
                lhsT = xr_t[:, i * P:(i + 1) * P]
                cs = i * P
                for j in range(LEV):
                    ps = psp.tile([P, 2 * FH], dt.float32, tag="ps")
                    for o in range(0, w, FH):
                        pw = min(FH, w - o)
                        nc.tensor.matmul(
                            ps[:, o:o + pw],
                            lhsT,
                            xm_t[:, j * N + cs + o:j * N + cs + o + pw],
                            start=True,
                            stop=True,
                        )
                    if MM_ENG[i] == "v":
                        nc.vector.tensor_copy(sc[:, j, :], ps[:, :w])
                    else:
                        nc.scalar.copy(sc[:, j, :], ps[:, :w])
                nc.sync.dma_start(outs[i][:], sc[:])
    nc.compile()
    return nc


def _get_nc(variant):
    if variant not in _nc_cache:
        _nc_cache[variant] = build_bass(variant)
    return _nc_cache[variant]


def host_inputs(evecs, variant="chain4"):
    """Per-core input maps. Core c -> (b=c//2, half=c%2)."""
    in_maps = []
    for c in range(NCORES):
        b, half = divmod(c, 2)
        X = np.asarray(evecs[b, 0], dtype=np.float32)      # [1024, 16]
        xr16 = np.ascontiguousarray(X.T).astype(np.float16)  # [16, 1024]
        xm16 = np.zeros((KC, LEV, N), np.float16)
        for j in range(LEV):
            kmax = half * LEV + j + 1
            xm16[:kmax, j, :] = xr16[:kmax]
        yb16 = np.zeros((1, LEV, FH), np.float16)
        xc32 = np.zeros((P, 4 * LEV), np.float32)
        for j in range(LEV):
            g = half * LEV + j
            yb16[0, j, :] = xr16[g, FH:N]
            for i in CHAIN_CHUNKS:
                xc32[:, (i - 4) * LEV + j] = X[i * P:(i + 1) * P, g]
        in_maps.append({
            "xr": xr16,
            "xm": np.ascontiguousarray(xm16.reshape(KC, LEV * N)),
            "yb": np.ascontiguousarray(yb16.reshape(1, LEV * FH)),
            "xc": xc32,
        })
    return in_maps


def unpack(results):
    """Assemble the full fp32 output from per-core packed fp16 buffers."""
    full = np.empty((B, NLEV, N, N), np.float32)
    for c in range(NCORES):
        b, half = divmod(c, 2)
        for i in range(RC):
            blk = results[c][f"out{i}"]   # [P, LEV, w]
            for j in range(LEV):
                slab = full[b, half * LEV + j]
                slab[i * P:(i + 1) * P, i * P:] = blk[:, j, :]
    V = full.reshape(B, NLEV, RC, P, RC, P)
    for i2 in range(RC):
        for j2 in range(i2):
            V[:, :, i2, :, j2, :] = V[:, :, j2, :, i2, :].swapaxes(-2, -1)
    return full


def run(evecs, trace=False, mm_dtype="chain4", **spmd_kwargs):
    from concourse.bass_utils import run_bass_kernel_spmd

    variant = "chain4"
    nc = _get_nc(variant)
    in_maps = host_inputs(evecs, variant)
    r = run_bass_kernel_spmd(
        nc, in_maps, core_ids=list(range(NCORES)), trace=trace, **spmd_kwargs
    )
    return unpack(r.results), r


def kernel(**inputs):
    evecs = np.asarray(inputs["evecs"])
    full, _ = run(evecs)
    return full
